# revision 5
# baseline (speedup 1.0000x reference)
"""Trainium2 Bass kernel for nn_DentalAnatomyLoss.

Same d-major layout as v2 (contiguous 32 KiB DMA lines, SWDGE cast to
bf16), but rebalanced around two measured facts:
  - any DVE op with accum_out runs at 1x (fused sum ops are 1 elem/cyc);
  - plain tensor_tensor (bf16, aligned) runs at 2x.

So dy = tensor_tensor(max) at 2x (output is non-negative), and its SUM
is offloaded: half the groups sum on TensorE (ones-column matmul
accumulated into spare selector-PSUM rows), half on ScalarE (Abs+accum).
dx keeps the fused 1x STT-max (a separate sum pass would cost more).
The bidiag lhsT's two spare columns (d-1, 2d-1) carry a ones-vector
(per-tile totals) and the crownA selector, so their sums ride the
existing PSUM drains for free; a tiny [P,2] selector matmul adds rootA
and crownB, and rootB falls out by subtraction.

Engine budget per core (approximate, measured rates):
  DMA 99.4us | DVE ~106 | ACT ~104 | PE ~104  -> ~107us target
"""

import os

import numpy as np

B, C, D, H, W = 2, 32, 64, 128, 128
NCORES = 8
JPC = (B * C) // NCORES
CROWN_ROOT_W = 2.0
SMOOTH_W = 1.5
EXPECTED_RATIO = 1.2

_PROG_CACHE: dict = {}
last_exec_time_ns = None


def _layout(jpc, d, h, w):
    P = 2 * d
    NG = jpc // 2
    NT = 2 * NG
    ROWS = h // 2
    FH = ROWS * w
    NDR = FH // 1024  # drains per tile
    pe_groups = set(range(NG))  # dy-sums ride PE (matmuls are cheap)
    cols = {}
    off = 0
    for name, n in (
        ("DX", NT),
        ("DY", NT),
        ("C0", NT),
        ("C127", NT),
        ("R0", NT),
        ("R63", NT),
        ("BND", NG),
        ("SELG", NG),
        ("DZ", NT * NDR),
    ):
        cols[name] = off
        off += n
    return P, NG, NT, ROWS, FH, NDR, pe_groups, cols, off


def _build_program(jpc=JPC, d=D, h=H, w=W, repeat=1, skip=(), unroll=1):
    from contextlib import ExitStack

    import concourse.tile as tile
    from concourse import bacc, mybir

    f32 = mybir.dt.float32
    bf16 = mybir.dt.bfloat16
    int8 = mybir.dt.int8
    AO = mybir.AluOpType
    AF = mybir.ActivationFunctionType

    P, NG, NT, ROWS, FH, NDR, PEG, COLS, NCOL = _layout(jpc, d, h, w)
    assert FH % 1024 == 0 and P <= 128

    nc = bacc.Bacc(
        "TRN2",
        target_bir_lowering=False,
        debug=False,
        enable_asserts=False,
        num_devices=NCORES,
    )
    seg = nc.dram_tensor("seg", [jpc * d, h * w], f32, kind="ExternalInput").ap()
    bd = nc.dram_tensor("bidiag", [P, P], bf16, kind="ExternalInput").ap()
    sel = nc.dram_tensor("sel", [P, 2], bf16, kind="ExternalInput").ap()
    ones = nc.dram_tensor("ones", [P, 1], bf16, kind="ExternalInput").ap()
    out = nc.dram_tensor("partials", [128, NCOL], f32, kind="ExternalOutput").ap()

    with tile.TileContext(nc) as tc, ExitStack() as ctx:
        singles = ctx.enter_context(tc.tile_pool(name="singles", bufs=1))
        xbp = ctx.enter_context(tc.tile_pool(name="xb", bufs=8))
        dyp = ctx.enter_context(tc.tile_pool(name="dy", bufs=2))
        dump = ctx.enter_context(tc.tile_pool(name="dump", bufs=1))
        psp = ctx.enter_context(tc.tile_pool(name="ps", bufs=3, space="PSUM"))
        selp = ctx.enter_context(tc.tile_pool(name="selps", bufs=2, space="PSUM"))

        bd_sb = singles.tile([P, P], bf16)
        nc.sync.dma_start(out=bd_sb, in_=bd)
        sel_sb = singles.tile([P, 2], bf16)
        nc.sync.dma_start(out=sel_sb, in_=sel)
        ones_sb = singles.tile([P, 1], bf16)
        nc.sync.dma_start(out=ones_sb, in_=ones)
        acc = singles.tile([128, NCOL], f32)
        nc.vector.memset(acc, 0.0)

        dx_out = dump.tile([P, ROWS, w - 1], int8)
        sm_out = dump.tile([P, w], bf16)
        sc_out = dump.tile([P, ROWS, 1], bf16)

        def ac(name, i):
            c = COLS[name] + i
            return acc[0:P, c : c + 1]

        # dy_out block boundaries for the PE ones-matmuls: uniform blocks
        # (<= 512, the matmul free-dim cap) so every accumulating matmul
        # covers the same PSUM cells.
        nblk_dy = 2 * NDR  # 16 blocks
        blen_dy = (FH - w) // nblk_dy  # 504
        assert blen_dy * nblk_dy == FH - w and blen_dy <= 512

        def body():
            state = {"prev": None, "selps": None, "pending_drain": None}
            for t in range(NT):
                g, hf = t // 2, t % 2
                xb = xbp.tile([P, FH], bf16)
                nc.gpsimd.dma_start(
                    out=xb, in_=seg[g * P : (g + 1) * P, hf * FH : (hf + 1) * FH]
                )
                x3 = xb.rearrange("p (r c) -> p r c", c=w)

                # --- dy: TT-max at 2x into a real buffer ---
                dy_out = dyp.tile([P, FH - w], bf16)
                if "dy" not in skip:
                    nc.vector.tensor_tensor(
                        out=dy_out, in0=xb[:, w:FH], in1=xb[:, 0 : FH - w],
                        op=AO.max,
                    )

                # --- dx: fused 1x STT-max (optimal: fused beats TT+sum) ---
                if "dx" not in skip:
                    nc.vector.scalar_tensor_tensor(
                        out=dx_out, in0=x3[:, :, 1:w], scalar=0.0,
                        in1=x3[:, :, 0 : w - 1],
                        op0=AO.bypass, op1=AO.max, accum_out=ac("DX", t),
                    )

                # --- smalls on ACT (Abs == identity for non-negative x) ---
                if "smalls" not in skip:
                    nc.scalar.activation(
                        out=sm_out, in_=xb[:, 0:w], func=AF.Abs,
                        accum_out=ac("R0", t),
                    )
                    nc.scalar.activation(
                        out=sm_out, in_=xb[:, FH - w : FH], func=AF.Abs,
                        accum_out=ac("R63", t),
                    )
                    nc.scalar.activation(
                        out=sc_out, in_=x3[:, :, 0:1], func=AF.Abs,
                        accum_out=ac("C0", t),
                    )
                    nc.scalar.activation(
                        out=sc_out, in_=x3[:, :, w - 1 : w], func=AF.Abs,
                        accum_out=ac("C127", t),
                    )
                    # dy boundary pair: fused STT-max (tiny)
                    if hf == 1:
                        nc.vector.scalar_tensor_tensor(
                            out=sm_out, in0=xb[:, 0:w], scalar=0.0,
                            in1=state["prev"][:, FH - w : FH],
                            op0=AO.bypass, op1=AO.max, accum_out=ac("BND", g),
                        )
                state["prev"] = xb

                # --- dz + sel matmuls (FD=512) + FD=1024 drains ---
                if "dz" not in skip:
                    if hf == 0:
                        sel_ps_new = selp.tile([33, 512], f32)
                        state["selps"] = sel_ps_new
                    sel_ps = state["selps"]
                    for k in range(NDR):
                        ps = psp.tile([P, 1024], f32)
                        for j in range(2):
                            blk = 2 * k + j
                            rhs = xb[:, blk * 512 : (blk + 1) * 512]
                            nc.tensor.matmul(
                                ps[:, j * 512 : (j + 1) * 512], bd_sb, rhs,
                                start=True, stop=True,
                            )
                            nc.tensor.matmul(
                                sel_ps[0:2, 0:512], sel_sb, rhs,
                                start=(hf == 0 and blk == 0),
                                stop=(hf == 1 and blk == 2 * NDR - 1),
                            )
                        nc.scalar.activation(
                            out=ps, in_=ps, func=AF.Abs,
                            accum_out=ac("DZ", t * NDR + k),
                        )

                # --- dy sums: PE groups via ones-matmul, others via ACT ---
                if "dy" not in skip:
                    if g in PEG and "dz" not in skip:
                        sel_ps = state["selps"]
                        for bi in range(nblk_dy):
                            nc.tensor.matmul(
                                sel_ps[32:33, 0:blen_dy],
                                ones_sb,
                                dy_out[:, bi * blen_dy : (bi + 1) * blen_dy],
                                start=(hf == 0 and bi == 0),
                                stop=(hf == 1 and bi == nblk_dy - 1),
                            )
                    else:
                        nc.scalar.activation(
                            out=dy_out, in_=dy_out, func=AF.Abs,
                            accum_out=ac("DY", t),
                        )

                # sel drains, deferred one group so ACT never waits on
                # this group's DVE/PE tail
                if hf == 1 and "dz" not in skip:
                    if state["pending_drain"] is not None:
                        _emit_sel_drain(*state["pending_drain"])
                    state["pending_drain"] = (g, state["selps"])
            if state["pending_drain"] is not None:
                _emit_sel_drain(*state["pending_drain"])
                state["pending_drain"] = None

        def _emit_sel_drain(g, sel_ps):
            cg = COLS["SELG"] + g
            nc.scalar.activation(
                out=sel_ps[0:2, :], in_=sel_ps[0:2, :], func=AF.Abs,
                accum_out=acc[0:2, cg : cg + 1],
            )
            if g in PEG and "dy" not in skip:
                nc.scalar.activation(
                    out=sel_ps[32:33, 0:blen_dy],
                    in_=sel_ps[32:33, 0:blen_dy],
                    func=AF.Abs,
                    accum_out=acc[32:33, cg : cg + 1],
                )

        # The For_i hardware loop carries an all-engine barrier per
        # iteration (~7us of pipeline refill).  Amortize it by emitting
        # several kernel bodies per loop iteration; total body count
        # stays exactly `repeat`.
        del unroll  # superseded by the fixed internal unroll below
        u = 8
        if repeat < 2 * u:
            for _ in range(repeat):
                body()
        else:
            with tc.For_i(0, repeat // u, 1):
                for _ in range(u):
                    body()
            for _ in range(repeat % u):
                body()
        nc.sync.dma_start(out=out, in_=acc)

    nc.compile()
    return nc


def _get_program():
    key = "full"
    if key not in _PROG_CACHE:
        _PROG_CACHE[key] = _build_program()
    return _PROG_CACHE[key]


def _bidiag_np(d=D):
    """Bidiag lhsT with spare columns carrying ones (d-1) and crownA (2d-1)."""
    import ml_dtypes

    P = 2 * d
    m = np.zeros((P, P), dtype=np.float32)
    for c in range(P - 1):
        if c == d - 1:
            continue
        m[c, c] = -1.0
        m[c + 1, c] = 1.0
    m[:, d - 1] = 1.0  # ones column -> per-tile total sums
    m[:, P - 1] = 0.0
    m[0 : d // 2, P - 1] = 1.0  # crownA selector
    return m.astype(ml_dtypes.bfloat16)


def _sel_np(d=D):
    """[P, 2] selector lhsT: rootA, crownB."""
    import ml_dtypes

    P = 2 * d
    m = np.zeros((P, 2), dtype=np.float32)
    m[d // 2 : d, 0] = 1.0  # rootA
    m[d : d + d // 2, 1] = 1.0  # crownB
    return m.astype(ml_dtypes.bfloat16)


def _ones_np(d=D):
    import ml_dtypes

    return np.ones((2 * d, 1), dtype=np.float32).astype(ml_dtypes.bfloat16)


def _combine(partials, jpc=JPC, d=D, h=H, w=W):
    P, NG, NT, ROWS, FH, NDR, PEG, COLS, NCOL = _layout(jpc, d, h, w)
    nslice = jpc * len(partials)

    crown = np.zeros(nslice, dtype=np.float64)
    root = np.zeros(nslice, dtype=np.float64)
    gx_sum = gy_sum = gz_sum = 0.0
    for ki, p in enumerate(partials):
        p64 = p.astype(np.float64)

        def block(name, n):
            c = COLS[name]
            return p64[:, c : c + n]

        DXs = block("DX", NT).sum(axis=0)
        DYs = block("DY", NT).sum(axis=0)
        C0s = block("C0", NT).sum(axis=0)
        C127s = block("C127", NT).sum(axis=0)
        R0s = block("R0", NT).sum(axis=0)
        R63s = block("R63", NT).sum(axis=0)
        BNDs = block("BND", NG).sum(axis=0)
        DZb = block("DZ", NT * NDR)  # [128, NT*NDR]
        # selg rows: 0=rootA, 1=crownB, 2=PE dy-sum (acc row 32)
        selg = p64[[0, 1, 32], COLS["SELG"] : COLS["SELG"] + NG]

        # per-tile totals from the ones column (row d-1); crownA from row 2d-1
        Tt = DZb[d - 1].reshape(NT, NDR).sum(axis=1)
        crownA_g = DZb[P - 1].reshape(NG, 2 * NDR).sum(axis=1)
        mask = np.ones(128, dtype=bool)
        mask[d - 1] = False
        mask[P - 1] = False
        mask[P:] = False
        gz_sum += DZb[mask].sum()

        T_all = Tt.sum()
        gx_sum += 2.0 * DXs.sum() - (2.0 * T_all - C0s.sum() - C127s.sum())

        # dy: per-tile sums; PE groups read selg row 2, others the DY cols
        for g in range(NG):
            t0, t1 = 2 * g, 2 * g + 1
            if g in PEG:
                dy_pair = selg[2, g]
            else:
                dy_pair = DYs[t0] + DYs[t1]
            corr = (2.0 * Tt[t0] - R0s[t0] - R63s[t0]) + (
                2.0 * Tt[t1] - R0s[t1] - R63s[t1]
            )
            gy_sum += 2.0 * dy_pair - corr
            gy_sum += 2.0 * BNDs[g] - R0s[t1] - R63s[t0]

            rootA = selg[0, g]
            crownB = selg[1, g]
            T_g = Tt[t0] + Tt[t1]
            cA = crownA_g[g]
            rB = T_g - cA - rootA - crownB
            crown[ki * jpc + 2 * g] = cA
            root[ki * jpc + 2 * g] = rootA
            crown[ki * jpc + 2 * g + 1] = crownB
            root[ki * jpc + 2 * g + 1] = rB

    total = crown + root
    valid = (total > 0) & (root > 0)
    safe_root = np.where(root > 0, root, 1.0)
    ratio_loss = np.where(valid, (crown / safe_root - EXPECTED_RATIO) ** 2, 0.0)
    cr_loss = ratio_loss.sum() / nslice

    nx = nslice * d * h * (w - 1)
    ny = nslice * d * (h - 1) * w
    nz = nslice * (d - 1) * h * w
    tv = gx_sum / nx + gy_sum / ny + gz_sum / nz

    crown_root = cr_loss * CROWN_ROOT_W
    smoothness = tv * SMOOTH_W
    return np.array(
        [crown_root, smoothness, crown_root + smoothness], dtype=np.float32
    )


def _shard_in_maps(seg_flat):
    bd = _bidiag_np()
    sl = _sel_np()
    on = _ones_np()
    return [
        {
            "seg": np.ascontiguousarray(
                seg_flat[k * JPC : (k + 1) * JPC].reshape(JPC * D, H * W)
            ),
            "bidiag": bd,
            "sel": sl,
            "ones": on,
        }
        for k in range(NCORES)
    ]


def _timing_in_maps():
    rng = np.random.default_rng(0)
    seg = rng.random((B * C, D, H * W), dtype=np.float32)
    return _shard_in_maps(seg)


def kernel(segmentation: np.ndarray) -> np.ndarray:
    global last_exec_time_ns
    from concourse.bass_utils import run_bass_kernel_spmd

    seg = np.ascontiguousarray(np.asarray(segmentation), dtype=np.float32)
    assert seg.shape == (B, C, D, H, W)
    nc = _get_program()
    in_maps = _shard_in_maps(seg.reshape(B * C, D, H * W))
    trace = bool(os.environ.get("BASS_TRACE"))
    res = run_bass_kernel_spmd(nc, in_maps, list(range(NCORES)), trace=trace)
    last_exec_time_ns = res.exec_time_ns
    partials = [res.results[k]["partials"] for k in range(NCORES)]
    return _combine(partials)
